# revision 5
# baseline (speedup 1.0000x reference)
"""Trainium2 Bass kernel for nn_AttentionSimple (sparse_attention, 8 cores).

Reference (per batch row b):
    e      = embeddings[k[b]]              # [S, E] gather
    scores = q[b] . e[s]                   # [S]
    attn   = softmax(scores); ctx = sum_s attn[s] * e[s]
    out    = ctx @ W.T + b                 # [B, 2]

Algorithm: count-weighted vocab-space softmax — no per-token gathers.
Scores depend on s only through v = k[b, s], so group softmax terms by
vocabulary id:
    c[b, v]  = |{s : k[b, s] = v}|         (histogram of k, built on host
                                            during input sharding)
    l[b, v]  = q[b] . embeddings[v]        (dense PE matmul)
    A        = c * exp(l)
    out[b]   = (sum_v A[b,v] * EW[v]) / (sum_v A[b,v])
    with EW  = embeddings @ W.T + b        (parameter prepacking, host)

Sharding: padded vocabulary (53248 = 416 chunks of 128) split across 8
cores (52 chunks each); every core handles all 128 batch rows. Cores
return partial numerators/denominators; host sums and divides.

v2 layout (per core), tuned from the v1 trace:
  - embT: bf16 [100, 3328]; chunk PAIRS stacked on the contraction dim
    (emb dims of even chunk at partitions 0:50, odd at 50:100 — no
    64-alignment padding, so only real bytes move). 26 pair-columns of
    128 vocab ids each.
  - mm1: ps[128, 256] = etpair.T @ qw  (qw = block-diag [qT|qT] f32r,
    [100, 256]); 4 pairs fill a [128, 1024] PSUM tile (one "oct" =
    8 chunks, 2 banks).
  - ACT: le = exp(ps) 1024 wide, bf16 out (amortizes the fixed PSUM
    access latency over 2x the elements vs the v1 512-wide tiles).
  - count multiply: le *= counts(u8), one [128, 512] op per half-oct;
    even halves on DVE, odd halves on GpSimd (splits the elementwise
    bottleneck across both engines).
  - mm2: acc[9, 1024] += st9_half.T @ le_half (bf16, 512 cols), st9 =
    [EW c0..c3 | ones]; even halves accumulate into acc[:, 0:512],
    odd into acc[:, 512:1024].
  - DMA: et chunks on the Sync queue, ct chunks on the GpSimd queue,
    qw/st/output on the Scalar queue — parallel queues with
    small-first chunks so the PE is never descriptor-feed starved
    (v1 serialized everything on one queue and stalled twice).
  - PE warmup: 5 matmuls on a memset tile emitted first so the PE
    p-state ramp (0.65 -> 2.4 GHz over ~3us of busy time) completes
    before the first real matmul.
"""

import numpy as np

BATCH, SEQ, EMB, VOCAB, OUT = 128, 8192, 50, 50000, 2
N_CORES = 8
CSH = 52                         # vocab chunks per core
NCHUNK = CSH * N_CORES           # 416
VPAD = NCHUNK * 128              # 53248
VSH = CSH * 128                  # 6656
NPAIR = CSH // 2                 # 26 pair-columns of 128 ids
NHALF = CSH // 4                 # 13 half-octs (4 chunks = 512 le cols)
NQW = 2 * BATCH                  # 256 moving columns of mm1
ETP = 2 * EMB                    # 100 real contraction partitions

# octs: groups of 4 pairs (8 chunks); last oct has 2 pairs
OCT_PAIRS = [4, 4, 4, 4, 4, 4, 2]
NOCT = len(OCT_PAIRS)
# DMA chunking (in octs) for et (sync queue) and ct (gpsimd queue)
ET_GROUPS = [1, 1, 2, 3]
CT_GROUPS = [2, 2, 3]
# which half-octs run their count-multiply on gpsimd (rest on DVE)
POOL_HALVES = frozenset((3, 5, 7, 9, 11))
N_WARMUP = 5

_CACHE = {}


def _build_nc():
    from contextlib import ExitStack

    import concourse.mybir as mybir
    import concourse.tile as tile
    from concourse import bacc

    f32 = mybir.dt.float32
    f32r = mybir.dt.float32r
    bf16 = mybir.dt.bfloat16
    u8 = mybir.dt.uint8
    nc = bacc.Bacc("TRN2", target_bir_lowering=False, debug=False,
                   num_devices=N_CORES)

    embT_d = nc.dram_tensor("embT", [ETP, NPAIR * 128], bf16,
                            kind="ExternalInput")
    qw_d = nc.dram_tensor("qw", [ETP, NQW], bf16, kind="ExternalInput")
    st_d = nc.dram_tensor("st", [128, NHALF * 9], bf16,
                          kind="ExternalInput")
    ct_d = nc.dram_tensor("ct", [128, VSH], u8, kind="ExternalInput")
    o_d = nc.dram_tensor("o", [9, 1024], f32, kind="ExternalOutput")

    # precomputed slices per oct / half
    oct_pair0 = np.cumsum([0] + OCT_PAIRS).tolist()

    with tile.TileContext(nc) as tc, ExitStack() as ctx:
        const_p = ctx.enter_context(tc.tile_pool(name="const", bufs=1))
        et_p = ctx.enter_context(tc.tile_pool(name="etp", bufs=4))
        ct_p = ctx.enter_context(tc.tile_pool(name="ctp", bufs=3))
        le_p = ctx.enter_context(tc.tile_pool(name="le", bufs=4))
        ps_p = ctx.enter_context(tc.tile_pool(name="ps", bufs=3,
                                              space="PSUM"))
        acc_p = ctx.enter_context(tc.tile_pool(name="acc", bufs=1,
                                               space="PSUM"))
        fin_p = ctx.enter_context(tc.tile_pool(name="fin", bufs=1))

        # PE warmup: matmuls on a memset tile (gpsimd memset runs as soon
        # as the framework preamble frees the Pool queue) so the PE
        # p-state ramp starts ~3us before the first input-dependent matmul.
        wtile = const_p.tile([128, 512], bf16)
        nc.gpsimd.memset(wtile[:], 0.0)
        wps = ps_p.tile([128, 1024], f32, tag="ps")
        for _ in range(N_WARMUP):
            nc.tensor.matmul(wps[:, 0:512], lhsT=wtile[:, 0:128],
                             rhs=wtile[:], start=True, stop=True)

        # small operands first on the scalar queue: qw gates the first
        # real matmul
        qw_sb = const_p.tile([ETP, NQW], bf16)
        nc.scalar.dma_start(qw_sb[:], qw_d.ap())
        st_sb = const_p.tile([128, NHALF * 9], bf16)
        nc.scalar.dma_start(st_sb[:], st_d.ap())

        # et chunks on the sync queue
        et_tiles = []
        o0 = 0
        for gsz in ET_GROUPS:
            p0, p1 = oct_pair0[o0], oct_pair0[min(o0 + gsz, NOCT)]
            t = et_p.tile([ETP, 4 * 3 * 128], bf16, tag="et")
            nc.sync.dma_start(t[:, 0:(p1 - p0) * 128],
                              embT_d.ap()[:, p0 * 128:p1 * 128])
            et_tiles.append((o0, o0 + gsz, p0, t))
            o0 += gsz

        # ct chunks on the gpsimd queue
        ct_tiles = []
        o0 = 0
        for gsz in CT_GROUPS:
            c0 = oct_pair0[o0] * 256
            c1 = oct_pair0[min(o0 + gsz, NOCT)] * 256
            t = ct_p.tile([128, 3 * 2048], u8, tag="ct")
            nc.gpsimd.dma_start(t[:, 0:c1 - c0], ct_d.ap()[:, c0:c1])
            ct_tiles.append((o0, o0 + gsz, c0, t))
            o0 += gsz

        def et_slice(pair):
            for (oa, ob, p0, t) in et_tiles:
                if oct_pair0[oa] <= pair < oct_pair0[min(ob, NOCT)]:
                    c = (pair - p0) * 128
                    return t[:, c:c + 128]
            raise AssertionError(pair)

        def ct_slice(half):
            c0 = half * 512
            for (oa, ob, b0, t) in ct_tiles:
                ca = oct_pair0[oa] * 256
                cb = oct_pair0[min(ob, NOCT)] * 256
                if ca <= c0 < cb:
                    c = c0 - b0
                    return t[:, c:c + 512]
            raise AssertionError(half)

        acc = acc_p.tile([9, 1024], f32)
        le_tiles = [None] * NOCT
        acc_started = [False, False]

        # last accumulating half per bank (banks take alternating halves)
        last_half = {0: max(h for h in range(NHALF) if h % 2 == 0),
                     1: max(h for h in range(NHALF) if h % 2 == 1)}

        def emit_mm2(o):
            le = le_tiles[o]
            npair = OCT_PAIRS[o]
            for hh in range(npair // 2):
                half = oct_pair0[o] // 2 + hh
                bank = half % 2
                is_last = half == last_half[bank]
                nc.tensor.matmul(
                    acc[:, bank * 512:(bank + 1) * 512],
                    lhsT=st_sb[:, half * 9:(half + 1) * 9],
                    rhs=le[:, hh * 512:(hh + 1) * 512],
                    start=not acc_started[bank],
                    stop=bool(is_last),
                    skip_group_check=True,
                )
                acc_started[bank] = True

        for o in range(NOCT):
            npair = OCT_PAIRS[o]
            ps = ps_p.tile([128, 1024], f32, tag="ps")
            for lp in range(npair):
                pair = oct_pair0[o] + lp
                nc.tensor.matmul(
                    ps[:, lp * 256:(lp + 1) * 256],
                    lhsT=et_slice(pair),
                    rhs=qw_sb[:],
                    start=True, stop=True,
                )
            le = le_p.tile([128, 1024], bf16, tag="le")
            le_tiles[o] = le
            w = npair * 256
            nc.scalar.activation(le[:, 0:w], ps[:, 0:w],
                                 mybir.ActivationFunctionType.Exp)
            for hh in range(npair // 2):
                half = oct_pair0[o] // 2 + hh
                eng = nc.gpsimd if half in POOL_HALVES else nc.vector
                eng.tensor_mul(le[:, hh * 512:(hh + 1) * 512],
                               le[:, hh * 512:(hh + 1) * 512],
                               ct_slice(half))
            if o >= 2:
                emit_mm2(o - 2)
        emit_mm2(NOCT - 2)
        emit_mm2(NOCT - 1)

        osb = fin_p.tile([9, 1024], f32)
        nc.vector.tensor_copy(osb[:], acc[:])
        nc.scalar.dma_start(o_d.ap(), osb[:])

    nc.finalize()
    return nc


def _prep_inputs(q, k, embeddings, W, b):
    import ml_dtypes

    q = np.ascontiguousarray(q, dtype=np.float32)
    emb = np.ascontiguousarray(embeddings, dtype=np.float32)
    W = np.ascontiguousarray(W, dtype=np.float32)
    b = np.ascontiguousarray(b, dtype=np.float32)
    k = np.asarray(k)

    embT = np.zeros((EMB, VPAD), np.float32)
    embT[:, :VOCAB] = emb.T

    # mm1 moving operand: block-diagonal [qT | 0; 0 | qT], rows 0:50/50:100
    qw = np.zeros((ETP, NQW), ml_dtypes.bfloat16)
    qw[:EMB, 0:BATCH] = q.T
    qw[EMB:ETP, BATCH:2 * BATCH] = q.T

    # weight prepacking: EW = emb @ W.T + b (function of parameters only)
    EWp = np.zeros((VPAD, OUT), np.float32)
    EWp[:VOCAB] = emb @ W.T + b[None, :]

    flat = (np.arange(BATCH, dtype=np.int64)[:, None] * VPAD
            + k.astype(np.int64)).ravel()
    C = np.bincount(flat, minlength=BATCH * VPAD).reshape(BATCH, VPAD)
    assert C.max() <= 255, "count histogram overflows uint8 transport"

    in_maps = []
    for core in range(N_CORES):
        v0 = core * VSH
        blocks = embT[:, v0:v0 + VSH].reshape(EMB, CSH, 128)
        e2 = np.zeros((ETP, NPAIR, 128), np.float32)
        e2[:EMB] = blocks[:, 0::2, :]
        e2[EMB:ETP] = blocks[:, 1::2, :]
        e2 = np.ascontiguousarray(
            e2.reshape(ETP, NPAIR * 128)).astype(ml_dtypes.bfloat16)

        # st9 per half-oct: cols 2j+o = EW[chunk 4h+j, o]; col 8 = 1
        ew_blocks = EWp[v0:v0 + VSH].reshape(CSH, 128, OUT)  # [52, 128, 2]
        st = np.zeros((128, NHALF, 9), np.float32)
        for j in range(4):
            st[:, :, 2 * j:2 * j + 2] = (
                ew_blocks.reshape(NHALF, 4, 128, OUT)[:, j]
                .transpose(1, 0, 2))
        st[:, :, 8] = 1.0
        st = np.ascontiguousarray(
            st.reshape(128, NHALF * 9)).astype(ml_dtypes.bfloat16)

        ct = np.ascontiguousarray(
            C[:, v0:v0 + VSH].reshape(BATCH, CSH, 128)
            .transpose(2, 1, 0).reshape(128, CSH * BATCH)
            .astype(np.uint8))
        in_maps.append({"embT": e2, "qw": qw, "st": st, "ct": ct})
    return in_maps


def _run_device(in_maps, **kwargs):
    from concourse.bass_utils import run_bass_kernel_spmd

    if "nc" not in _CACHE:
        _CACHE["nc"] = _build_nc()
    return run_bass_kernel_spmd(_CACHE["nc"], in_maps,
                                core_ids=list(range(N_CORES)), **kwargs)


def _unshard(res):
    P = np.zeros((9, 1024), np.float64)
    for i in range(N_CORES):
        P += res.results[i]["o"].astype(np.float64)
    numer = np.zeros((OUT, BATCH), np.float64)
    denom = np.zeros(BATCH, np.float64)
    for bank in range(2):
        Pb = P[:, bank * 512:(bank + 1) * 512]
        for j in range(4):
            numer += Pb[2 * j:2 * j + 2, j * BATCH:(j + 1) * BATCH]
            denom += Pb[8, j * BATCH:(j + 1) * BATCH]
    out = (numer / denom[None, :]).T
    return np.ascontiguousarray(out, dtype=np.float32)


def kernel(q, k, embeddings, W, b, **_unused):
    in_maps = _prep_inputs(q, k, embeddings, W, b)
    res = _run_device(in_maps)
    return _unshard(res)
